# revision 16
# baseline (speedup 1.0000x reference)
"""Trainium2 Bass kernel for nn_AttentionSeqToMasked (dense transformer attention).

Full-input contract: kernel(**inputs) takes the unsharded numpy inputs and
returns the full [B, SQ, H*D_V] float32 output.

Sharding (8 cores): data parallel over batch (B=4 -> 2 cores per batch) x
tensor parallel over heads (16 heads -> 8 per core). Each core computes
attention for one (batch, head-half) pair; host gathers the slices.

Per-core dataflow (all matmuls bf16 inputs, fp32 PSUM accumulation):
  - Host pre-transposes activations to X^T [D_PRE, S] bf16 so the contraction
    dim (D_PRE) lands on SBUF partitions; loads are chopped into column
    chunks ordered by urgency (k projection feeds all 16 key tiles of the
    first query chunk, so xk streams right after the first xq chunk).
  - Projections compute q^T/k^T = W^T @ X^T directly (head-dim on partitions),
    v in natural [s, d_v] layout with a ones-column appended via the weight
    matrix (zero weight column + bias 1.0).
  - Scores are computed transposed: scoresT[k, q] = kT.T @ qT, two heads
    packed into the 128x128 PE array per matmul pair (d_head=64 row groups,
    which the PE runs concurrently when the weight-load port has slack).
  - Key-mask folds into the exp as a per-partition bias (0 or -30000);
    1/sqrt(d) folds into the exp scale.
  - AV matmul contracts exp(scores)T with [v | ones]: row 64 of the psum is
    the softmax denominator, computed for free alongside the numerator.
  - No on-device transpose/normalize: the [65, 512] numerator+denominator
    psum is copied to SBUF and DMA'd out as-is; the HOST divides by the
    denominator row and transposes when assembling the full output.

Scheduling: projection work for pair p+1 is chopped into ~0.85us psum-chunks
and spread evenly over the first 96 blocks of pair p's attention loop,
keeping TensorE fed while ScalarE (exp, ~1.04us per 128x1024 tile) runs
saturated.
"""

import os
from contextlib import ExitStack

import numpy as np
import ml_dtypes

import concourse.bass as bass
import concourse.bacc as bacc
import concourse.mybir as mybir
import concourse.tile as tile
from concourse.bass_utils import run_bass_kernel_spmd

# Problem shape (hardcoded per contract)
B, SQ, SK = 4, 2048, 2048
D_PRE = 1024
H, D_QK, D_V = 16, 64, 64
N_CORES = 8
HALF = (H // 2) * D_QK  # 512 columns of the projection handled per core
N_PAIRS = 4  # head pairs per core
S_CHUNK = 512  # moving free-dim per q-projection matmul
K_CHUNK = 256  # k-projection chunk width (fine-grained DMA dependencies)
N_DT = D_PRE // 128  # d_pre tiles of 128
N_KT = SK // 128  # key tiles of 128
N_QC = SQ // S_CHUNK  # query chunks of 512
N_KC = SK // K_CHUNK  # k-projection chunks of 256
MASK_NEG = -30000.0

F32 = mybir.dt.float32
BF16 = mybir.dt.bfloat16
BF16_NP = np.dtype(ml_dtypes.bfloat16)

OUT_ROWS = N_PAIRS * 130  # per (pair, head): 64 numerator rows + 1 denom row

_COMPILED = None


def _build_program():
    nc = bacc.Bacc("TRN2", target_bir_lowering=False, debug=False)

    xq = nc.dram_tensor("xq", [D_PRE, SQ], BF16, kind="ExternalInput").ap()
    xk = nc.dram_tensor("xk", [D_PRE, SK], BF16, kind="ExternalInput").ap()
    xv = nc.dram_tensor("xv", [D_PRE, SK], BF16, kind="ExternalInput").ap()
    wq = nc.dram_tensor("wq", [D_PRE, HALF], BF16, kind="ExternalInput").ap()
    wk = nc.dram_tensor("wk", [D_PRE, HALF], BF16, kind="ExternalInput").ap()
    # v weights with a zero column appended per head (ones column generator)
    wv = nc.dram_tensor("wv", [D_PRE, N_PAIRS * 130], BF16, kind="ExternalInput").ap()
    bq = nc.dram_tensor("bq", [128, N_PAIRS], F32, kind="ExternalInput").ap()
    bk = nc.dram_tensor("bk", [128, N_PAIRS], F32, kind="ExternalInput").ap()
    bv = nc.dram_tensor("bv", [128, N_PAIRS * 130], F32, kind="ExternalInput").ap()
    mb = nc.dram_tensor("mb", [128, N_KT], F32, kind="ExternalInput").ap()
    out = nc.dram_tensor("out", [OUT_ROWS, SQ], F32, kind="ExternalOutput").ap()

    with tile.TileContext(nc) as tc:
        _emit(tc, xq, xk, xv, wq, wk, wv, bq, bk, bv, mb, out)

    nc.compile()
    return nc


def _emit(tc, xq, xk, xv, wq, wk, wv, bq, bk, bv, mb, out):
    nc = tc.nc

    with ExitStack() as ctx:
        # ---- pools ----
        xp = ctx.enter_context(tc.tile_pool(name="x", bufs=1))
        wp = ctx.enter_context(tc.tile_pool(name="w", bufs=1))
        cp = ctx.enter_context(tc.tile_pool(name="const", bufs=1))
        qkvp = ctx.enter_context(tc.tile_pool(name="qkv", bufs=1))
        expp = ctx.enter_context(tc.tile_pool(name="exp", bufs=22))
        stgp = ctx.enter_context(tc.tile_pool(name="stg", bufs=3))

        proj_ps = ctx.enter_context(tc.tile_pool(name="proj_ps", bufs=2, space="PSUM"))
        sc_ps = ctx.enter_context(tc.tile_pool(name="sc_ps", bufs=2, space="PSUM"))
        av_ps = ctx.enter_context(tc.tile_pool(name="av_ps", bufs=2, space="PSUM"))

        # ---- constants ----
        mb_sb = cp.tile([128, N_KT], F32, name="mb_sb")
        nc.sync.dma_start(mb_sb, mb)
        bq_sb = cp.tile([128, N_PAIRS], F32, name="bq_sb")
        nc.sync.dma_start(bq_sb, bq)
        bk_sb = cp.tile([128, N_PAIRS], F32, name="bk_sb")
        nc.sync.dma_start(bk_sb, bk)
        bv_sb = cp.tile([128, N_PAIRS * 130], F32, name="bv_sb")
        nc.sync.dma_start(bv_sb, bv)

        # ---- streamed loads, column-chunked and ordered by urgency ----
        # Chunk columns so each projection matmul waits only on the bytes it
        # reads, while keeping DMA rows >= 1KB (descriptor efficiency) —
        # except a tiny [*, 0:128] xk chunk that unblocks the very first
        # scores tile. Order matches the consumption deadlines.
        x_chunks = {
            "xq": [(0, 512), (512, 1024), (1024, 2048)],
            "xk": [(0, 128), (128, 512), (512, 1024), (1024, 2048)],
            "xv": [(0, 1024), (1024, 2048)],
        }
        x_tiles = {}  # (pfx, chunk_idx) -> [8 dt tiles]

        def load_x_chunk(xap, pfx, ci):
            c0, c1 = x_chunks[pfx][ci]
            ts = []
            for dt_i in range(N_DT):
                t = xp.tile(
                    [128, c1 - c0], BF16, name=f"{pfx}{ci}_{dt_i}", tag=f"{pfx}{ci}_{dt_i}"
                )
                nc.sync.dma_start(t, xap[dt_i * 128 : (dt_i + 1) * 128, c0:c1])
                ts.append(t)
            x_tiles[(pfx, ci)] = ts

        def x_slice(pfx, c0, width):
            # tiles covering columns [c0, c0+width) (must lie in one chunk)
            for ci, (lo, hi) in enumerate(x_chunks[pfx]):
                if lo <= c0 and c0 + width <= hi:
                    ts = x_tiles[(pfx, ci)]
                    return [t[:, c0 - lo : c0 - lo + width] for t in ts]
            raise AssertionError((pfx, c0, width))

        def load_w(wap, pfx, width):
            ts = []
            for dt_i in range(N_DT):
                t = wp.tile([128, width], BF16, name=f"{pfx}{dt_i}", tag=f"{pfx}{dt_i}")
                nc.sync.dma_start(t, wap[dt_i * 128 : (dt_i + 1) * 128, :])
                ts.append(t)
            return ts

        wq_sb = load_w(wq, "wq", HALF)
        load_x_chunk(xq, "xq", 0)
        wk_sb = load_w(wk, "wk", HALF)
        load_x_chunk(xk, "xk", 0)
        load_x_chunk(xk, "xk", 1)
        load_x_chunk(xk, "xk", 2)
        load_x_chunk(xk, "xk", 3)
        load_x_chunk(xq, "xq", 1)
        wv_sb = load_w(wv, "wv", N_PAIRS * 130)
        load_x_chunk(xv, "xv", 0)
        load_x_chunk(xv, "xv", 1)
        load_x_chunk(xq, "xq", 2)

        v_tiles = {}  # (pair, kt) -> [128, 130] bf16 tile
        qkT = {}  # (pfx, pair) -> [128, SQ] bf16 tile

        def qk_tile(pfx, pair):
            if (pfx, pair) not in qkT:
                qkT[(pfx, pair)] = qkvp.tile(
                    [128, SQ], BF16, name=f"{pfx}T{pair}", tag=f"{pfx}T", bufs=2
                )
            return qkT[(pfx, pair)]

        proj_ps_open = {}

        # (pfx, pair) projection chunk spans, matching the xk DMA chunking
        PROJ_SPANS = {
            "q": [(0, 512), (512, 1024), (1024, 1536), (1536, 2048)],
            "k": [(0, 128), (128, 512), (512, 1024), (1024, 1536), (1536, 2048)],
        }

        def emit_qk_chunk(pair, pfx, sp, half=None):
            # one projection chunk (<=512 cols): 8 accumulating MMs + bias
            # add. half=0/1 emits only the first/second 4 contraction MMs.
            c0, c1 = PROJ_SPANS[pfx][sp]
            w = c1 - c0
            dst = qk_tile(pfx, pair)
            w_sb = wq_sb if pfx == "q" else wk_sb
            b_sb = bq_sb if pfx == "q" else bk_sb
            xs = x_slice("x" + pfx, c0, w)
            key = (pair, pfx, sp)
            if half == 1:
                ps = proj_ps_open.pop(key)
            else:
                ps = proj_ps.tile(
                    [128, S_CHUNK], F32, name=f"{pfx}ps{pair}_{sp}", tag="proj"
                )
            dts = range(N_DT) if half is None else range(half * 4, half * 4 + 4)
            for dt_i in dts:
                nc.tensor.matmul(
                    ps[:, 0:w],
                    lhsT=w_sb[dt_i][:, pair * 128 : (pair + 1) * 128],
                    rhs=xs[dt_i],
                    start=(dt_i == 0),
                    stop=(dt_i == N_DT - 1),
                )
            if half == 0:
                proj_ps_open[key] = ps
            else:
                nc.vector.tensor_scalar_add(
                    dst[:, c0:c1],
                    ps[:, 0:w],
                    b_sb[:, pair : pair + 1],
                )

        def emit_v_chunk(g, st):
            # v projection for pairs (2g, 2g+1), one key tile: N=260 matmuls
            ps = proj_ps.tile([128, S_CHUNK], F32, name=f"vps{g}_{st}", tag="proj")
            xs = x_slice("xv", st * 128, 128)
            for dt_i in range(N_DT):
                nc.tensor.matmul(
                    ps[:, 0:260],
                    lhsT=xs[dt_i],
                    rhs=wv_sb[dt_i][:, g * 260 : (g + 1) * 260],
                    start=(dt_i == 0),
                    stop=(dt_i == N_DT - 1),
                )
            for j in range(2):
                pair = 2 * g + j
                vt = qkvp.tile(
                    [128, 130], BF16, name=f"v{pair}_{st}", tag="v", bufs=4 * N_KT
                )
                nc.vector.tensor_add(
                    vt,
                    ps[:, j * 130 : (j + 1) * 130],
                    bv_sb[:, pair * 130 : (pair + 1) * 130],
                )
                v_tiles[(pair, st)] = vt

        # filler queue: projection chunk units (~0.85us of PE each), spread
        # evenly over the attention stream. Ordering: qk(p) before pair-p
        # attention, v(g) before pair-2g attention.
        filler = []

        def _qk_half(pair, pfx, sp, half):
            return lambda: emit_qk_chunk(pair, pfx, sp, half)

        def add_pair_filler(pair):
            for sp in range(len(PROJ_SPANS["q"])):
                filler.append(_qk_half(pair, "q", sp, 0))
                filler.append(_qk_half(pair, "q", sp, 1))
            for sp in range(len(PROJ_SPANS["k"])):
                filler.append(lambda pair=pair, sp=sp: emit_qk_chunk(pair, "k", sp))

        add_pair_filler(1)
        filler += [(lambda st=st: emit_v_chunk(1, st)) for st in range(N_KT)]
        add_pair_filler(2)
        add_pair_filler(3)

        def pop_filler():
            if filler:
                filler.pop(0)()

        # prologue: pair-0 projections, ordered so the first scores tiles
        # unblock as soon as their DMA chunks land
        emit_qk_chunk(0, "q", 0)
        for sp in range(len(PROJ_SPANS["k"])):
            emit_qk_chunk(0, "k", sp)
        for sp in range(1, len(PROJ_SPANS["q"])):
            emit_qk_chunk(0, "q", sp)
        for st in range(N_KT):
            emit_v_chunk(0, st)

        # ---- software-pipelined attention stream over (pair, qc, kt) ----
        iters = [
            (pair, qc, kt)
            for pair in range(N_PAIRS)
            for qc in range(N_QC)
            for kt in range(N_KT)
        ]
        sc_map = {}
        av_map = {}
        ex_map = {}

        def emit_scores(i):
            pair, qc, kt = iters[i]
            qT = qk_tile("q", pair)
            kT = qk_tile("k", pair)
            sc = sc_ps.tile([128, 1024], F32, name=f"sc{pair}_{qc}_{kt}", tag="sc")
            # scoresT for heads A and B, packed in PE row groups (concurrent)
            nc.tensor.matmul(
                sc[:, 0:512],
                lhsT=kT[0:64, kt * 128 : (kt + 1) * 128],
                rhs=qT[0:64, qc * S_CHUNK : (qc + 1) * S_CHUNK],
                start=True,
                stop=True,
            )
            nc.tensor.matmul(
                sc[:, 512:1024],
                lhsT=kT[64:128, kt * 128 : (kt + 1) * 128],
                rhs=qT[64:128, qc * S_CHUNK : (qc + 1) * S_CHUNK],
                start=True,
                stop=True,
            )
            sc_map[i] = sc

        def emit_epilogue(pair, qc, av_a, av_b):
            # numerator rows 0:64 + denominator row 64, straight to DRAM;
            # the host normalizes and transposes. DMA split across 2 queues.
            for h_i, av in enumerate((av_a, av_b)):
                stg = stgp.tile(
                    [65, S_CHUNK], F32, name=f"st{pair}_{qc}_{h_i}", tag="stg"
                )
                nc.vector.tensor_copy(stg, av)
                base = pair * 130 + h_i * 65
                for lo, hi in ((0, 17), (17, 33), (33, 49), (49, 65)):
                    nc.sync.dma_start(
                        out[base + lo : base + hi, qc * S_CHUNK : (qc + 1) * S_CHUNK],
                        stg[lo:hi, :],
                    )

        def emit_av(pair, qc, kt, ex):
            if kt == 0:
                av_map[(pair, qc)] = (
                    av_ps.tile([65, S_CHUNK], F32, name=f"ava{pair}_{qc}", tag="av"),
                    av_ps.tile([65, S_CHUNK], F32, name=f"avb{pair}_{qc}", tag="av"),
                )
            av_a, av_b = av_map[(pair, qc)]
            nc.tensor.matmul(
                av_a,
                lhsT=v_tiles[(pair, kt)][:, 0:65],
                rhs=ex[:, 0:512],
                start=(kt == 0),
                stop=(kt == N_KT - 1),
            )
            nc.tensor.matmul(
                av_b,
                lhsT=v_tiles[(pair, kt)][:, 65:130],
                rhs=ex[:, 512:1024],
                start=(kt == 0),
                stop=(kt == N_KT - 1),
            )
            if kt == N_KT - 1:
                emit_epilogue(pair, qc, *av_map.pop((pair, qc)))

        # Emission in 2-iteration blocks, software-pipelined:
        #   block b: exps (2b, 2b+1) | AV burst (2b-2, 2b-1) | scores (2b+2,
        #   2b+3) | filler. Iters 0..21 (pair 0, into qc 1) defer their AVs
        #   so ScalarE starts while the v projection still waits on the xv
        #   DMA; the deferred AVs drain 4-per-block over blocks 11-15.
        emit_scores(0)
        emit_scores(1)
        n_it = len(iters)

        def emit_exp(i):
            pair, qc, kt = iters[i]
            ex = expp.tile([128, 1024], BF16, name=f"ex{pair}_{qc}_{kt}", tag="ex")
            ex_map[i] = ex
            nc.scalar.activation(
                ex,
                sc_map.pop(i),
                mybir.ActivationFunctionType.Exp,
                bias=mb_sb[:, kt : kt + 1],
                scale=0.125,
            )

        def emit_av_i(i):
            pair, qc, kt = iters[i]
            emit_av(pair, qc, kt, ex_map.pop(i))

        AV_DEFER = 13  # first block that runs AVs; needs xv landed
        next_av = 0
        for b in range(n_it // 2):
            i0, i1 = 2 * b, 2 * b + 1
            if b >= AV_DEFER:
                # catch up 4 AV iters per block (tracks the xv DMA landing)
                # until at steady-state lag (AVs through iter 2b-1)
                lim = min(i0 - 1, next_av + 3)
                while next_av <= lim:
                    emit_av_i(next_av)
                    next_av += 1
            emit_exp(i0)
            emit_exp(i1)
            if i1 + 2 < n_it:
                emit_scores(i1 + 1)
                emit_scores(i1 + 2)
            if b < 63:
                if b % 4 != 3:
                    pop_filler()
            elif b < 96 and b % 2 == 1:
                pop_filler()
        while next_av < n_it:
            emit_av_i(next_av)
            next_av += 1

        assert not filler, f"{len(filler)} filler chunks left unscheduled"
        assert not ex_map and not av_map and not sc_map


def _prep_core_inputs(pre_qs, pre_ks, pre_vs, k_mask, q_w, q_b, k_w, k_b, v_w, v_b, core):
    b = core // 2
    hh = core % 2
    cols = slice(HALF * hh, HALF * (hh + 1))

    xq = np.ascontiguousarray(pre_qs[b].T).astype(BF16_NP)
    xk = np.ascontiguousarray(pre_ks[b].T).astype(BF16_NP)
    xv = np.ascontiguousarray(pre_vs[b].T).astype(BF16_NP)
    wq = np.ascontiguousarray(q_w[:, cols]).astype(BF16_NP)
    wk = np.ascontiguousarray(k_w[:, cols]).astype(BF16_NP)

    wv_core = v_w[:, cols].astype(np.float32)
    wv = np.zeros((D_PRE, N_PAIRS * 130), dtype=np.float32)
    bv_core = v_b[cols].astype(np.float32)
    bv_ext = np.zeros(N_PAIRS * 130, dtype=np.float32)
    for p in range(N_PAIRS):
        wv[:, p * 130 : p * 130 + 64] = wv_core[:, p * 128 : p * 128 + 64]
        wv[:, p * 130 + 65 : p * 130 + 129] = wv_core[:, p * 128 + 64 : p * 128 + 128]
        bv_ext[p * 130 : p * 130 + 64] = bv_core[p * 128 : p * 128 + 64]
        bv_ext[p * 130 + 64] = 1.0
        bv_ext[p * 130 + 65 : p * 130 + 129] = bv_core[p * 128 + 64 : p * 128 + 128]
        bv_ext[p * 130 + 129] = 1.0

    bq = np.ascontiguousarray(q_b[cols].astype(np.float32).reshape(N_PAIRS, 128).T)
    bk = np.ascontiguousarray(k_b[cols].astype(np.float32).reshape(N_PAIRS, 128).T)
    bv_full = np.ascontiguousarray(np.tile(bv_ext[None, :], (128, 1)))

    # mask True -> 0.0, False -> MASK_NEG
    mbias = np.where(k_mask[b], 0.0, MASK_NEG).astype(np.float32)
    mb = np.ascontiguousarray(mbias.reshape(N_KT, 128).T)

    return {
        "xq": xq,
        "xk": xk,
        "xv": xv,
        "wq": wq,
        "wk": wk,
        "wv": wv.astype(BF16_NP),
        "bq": bq,
        "bk": bk,
        "bv": bv_full,
        "mb": mb,
    }


def kernel(pre_qs, pre_ks, pre_vs, k_mask, q_w, q_b, k_w, k_b, v_w, v_b):
    global _COMPILED
    args = (pre_qs, pre_ks, pre_vs, k_mask, q_w, q_b, k_w, k_b, v_w, v_b)
    args = tuple(np.asarray(a) for a in args)

    if _COMPILED is None:
        _COMPILED = _build_program()
    nc = _COMPILED

    in_maps = [_prep_core_inputs(*args, core=c) for c in range(N_CORES)]

    trace = bool(int(os.environ.get("BASS_KERNEL_TRACE", "0")))
    res = run_bass_kernel_spmd(
        nc,
        in_maps,
        core_ids=list(range(N_CORES)),
        trace=trace,
    )
    if trace:
        kernel.last_results = res

    out = np.empty((B, SQ, H * D_V), dtype=np.float32)
    for c in range(N_CORES):
        b = c // 2
        hh = c % 2
        raw = res.results[c]["out"]  # [N_PAIRS*130, SQ]: numer rows + denom row
        for p in range(N_PAIRS):
            for h_i in range(2):
                base = p * 130 + h_i * 65
                num = raw[base : base + 64]  # [64, SQ]
                den = raw[base + 64]  # [SQ]
                col = HALF * hh + p * 128 + h_i * 64
                out[b, :, col : col + 64] = (num / den).T
    return out
